# revision 1
# baseline (speedup 1.0000x reference)
"""MHSA (dense transformer, ALiBi + causal) TRN2 Bass kernel, 8-core SPMD.

Sharding: batch (2) x head-group (4 groups of 4 heads) -> 8 cores.
Each core computes, for its batch b and heads [4g, 4g+4):
  Q^T = (Wq_slice/8) @ X_q^T      [64+aug, 2048] per head   (f32r matmuls)
  K^T = Wk_slice @ X_k^T          [64+aug, 2048] per head
  V   = X_v @ Wv_slice^T          [2048, 64(+ones)] per head
  S^T = K~ @ Q~^T  with ALiBi folded in via 2 augmented contraction rows:
        Q~ = [Q, slope_h, -slope_h*i],  K~ = [K, j, 1]
  P~^T = exp(S^T - 12) (softmax shift is row-invariant; -12 guards overflow)
  causal: upper-triangle blocks skipped structurally; diagonal 128-strips
        masked by multiplying with a lower-triangular 0/1 tile.
  O^T  = V~^T @ P~^T ; V~ has a ones column so row 64 of O^T is the softmax
        denominator. PE-transpose O^T chunks, divide via scalar-engine copy
        with per-partition reciprocal scale, DMA out.

All matmul operands are float32r (full f32 bytes, 1 cyc/row on PE for N>=256).
"""

import numpy as np

import concourse.bass as bass
import concourse.mybir as mybir
import concourse.tile as tile
from concourse import bacc
from concourse.bass_utils import run_bass_kernel_spmd
from concourse.masks import make_identity

P = 128
S = 2048
D = 1024
H = 16
HWID = 64
HPC = 4           # heads per core
CW = HPC * HWID   # 256 output cols per core
NTSB = 8          # t super-blocks for projection streaming
TSB = S // NTSB   # 256
NKC = D // P      # 8 contraction chunks
NI = S // 512     # 4 i-chunks of 512
NJ = S // P       # 16 j-chunks of 128

F32 = mybir.dt.float32
F32R = mybir.dt.float32r

EXP_BIAS = -12.0


def build_kernel():
    nc = bacc.Bacc("TRN2")

    qT = nc.dram_tensor("qT", [D, S], F32R, kind="ExternalInput")
    kT = nc.dram_tensor("kT", [D, S], F32R, kind="ExternalInput")
    vT = nc.dram_tensor("vT", [D, S], F32R, kind="ExternalInput")
    wqT = nc.dram_tensor("wqT", [D, CW], F32R, kind="ExternalInput")
    wkT = nc.dram_tensor("wkT", [D, CW], F32R, kind="ExternalInput")
    wvT = nc.dram_tensor("wvT", [D, CW], F32R, kind="ExternalInput")
    aug = nc.dram_tensor("aug", [4 * HPC, S], F32R, kind="ExternalInput")
    tri = nc.dram_tensor("tri", [P, P], F32R, kind="ExternalInput")
    zs = nc.dram_tensor("zs", [62, S], F32R, kind="ExternalInput")
    on1 = nc.dram_tensor("on1", [P, 64], F32R, kind="ExternalInput")
    out = nc.dram_tensor("out", [S, CW], F32, kind="ExternalOutput")

    xT_drams = [qT, kT, vT]
    w_drams = [wqT, wkT, wvT]

    with tile.TileContext(nc) as tc:
        with (
            tc.tile_pool(name="wp", bufs=1) as wp,
            tc.tile_pool(name="xp", bufs=2) as xp,
            tc.tile_pool(name="qk", bufs=1) as qkp,
            tc.tile_pool(name="vv", bufs=1) as vvp,
            tc.tile_pool(name="pt", bufs=3) as ptp,
            tc.tile_pool(name="ot", bufs=2) as otp_sb,
            tc.tile_pool(name="ob", bufs=3) as obp,
            tc.tile_pool(name="rc", bufs=3) as rcp,
            tc.tile_pool(name="cn", bufs=1) as cnp,
        ):
            # ---- constants ----
            tri_t = cnp.tile([P, P], F32R, tag="tri", name="tri_t")
            nc.sync.dma_start(tri_t[:], tri[:])
            ident = cnp.tile([HWID + 1, HWID + 1], F32, tag="ident", name="ident")
            make_identity(nc, ident[:])
            ebias = cnp.tile([P, 1], F32, tag="ebias", name="ebias")
            nc.gpsimd.memset(ebias[:], EXP_BIAS)

            # ---- weights [P, NKC, CW] ----
            w_tiles = []
            for i, wd in enumerate(w_drams):
                wt = wp.tile([P, NKC, CW], F32R, tag=f"w{i}", name=f"w{i}")
                nc.sync.dma_start(wt[:], wd.rearrange("(ko p) c -> p ko c", p=P))
                w_tiles.append(wt)

            # ---- per-head Q~^T / K~^T tiles [128, S]; V~ [128, NJ, HPC, 65] ----
            # even local head: data rows 0:64, aug rows 64:66, matmul slice [0:66]
            # odd  local head: data rows 64:128, aug rows 62:64, slice [62:128]
            q_tiles = [qkp.tile([P, S], F32R, tag=f"qh{h}", name=f"qh{h}") for h in range(HPC)]
            k_tiles = [qkp.tile([P, S], F32R, tag=f"kh{h}", name=f"kh{h}") for h in range(HPC)]
            v_tile = vvp.tile([P, NJ, HPC, HWID + 1], F32R, tag="v", name="v")

            def aug_row(h):
                return HWID if h % 2 == 0 else HWID - 2

            def mm_slice(h):
                return slice(0, 66) if h % 2 == 0 else slice(0, 128)

            for h in range(HPC):
                ra = aug_row(h)
                nc.sync.dma_start(q_tiles[h][ra:ra + 2, :], aug[4 * h:4 * h + 2, :])
                nc.sync.dma_start(k_tiles[h][ra:ra + 2, :], aug[4 * h + 2:4 * h + 4, :])
            for h in range(1, HPC, 2):
                nc.sync.dma_start(q_tiles[h][0:62, :], zs[:])
                nc.sync.dma_start(k_tiles[h][0:62, :], zs[:])
            # ones column of V~
            nc.sync.dma_start(
                v_tile[:, :, :, HWID], on1.rearrange("p (a b) -> p a b", a=NJ)
            )

            # ================= projection phase =================
            with tc.tile_pool(name="pp", bufs=6, space="PSUM") as pp:
                for t in range(NTSB):
                    xt = xp.tile([P, NKC, 3, TSB], F32R, tag="x", name=f"x{t}")
                    for xi, xd in enumerate(xT_drams):
                        nc.sync.dma_start(
                            xt[:, :, xi, :],
                            xd.rearrange("(ko p) t -> p ko t", p=P)[
                                :, :, t * TSB:(t + 1) * TSB],
                        )
                    # Q^T, K^T: [128 (2 heads), TSB] accumulating over d
                    for pi, (wt, dsts) in enumerate(
                        [(w_tiles[0], q_tiles), (w_tiles[1], k_tiles)]
                    ):
                        for cc in range(2):
                            ps = pp.tile([P, TSB], F32, tag="pp", name=f"pp{t}_{pi}_{cc}")
                            for kk in range(NKC):
                                nc.tensor.matmul(
                                    ps[:],
                                    lhsT=wt[:, kk, cc * P:(cc + 1) * P],
                                    rhs=xt[:, kk, pi, :],
                                    start=(kk == 0),
                                    stop=(kk == NKC - 1),
                                )
                            # copyback: rows 0:64 -> head 2cc (rows 0:64),
                            #           rows 64:128 -> head 2cc+1 (rows 64:128)
                            nc.vector.tensor_copy(
                                dsts[2 * cc][0:HWID, t * TSB:(t + 1) * TSB],
                                ps[0:HWID, :],
                            )
                            nc.vector.tensor_copy(
                                dsts[2 * cc + 1][HWID:P, t * TSB:(t + 1) * TSB],
                                ps[HWID:P, :],
                            )
                    # V: [128 t, CW] per 128-t chunk
                    for u in range(TSB // P):
                        tt = t * (TSB // P) + u
                        ps = pp.tile([P, CW], F32, tag="pp", name=f"ppv{t}_{u}")
                        for kk in range(NKC):
                            nc.tensor.matmul(
                                ps[:],
                                lhsT=xt[:, kk, 2, u * P:(u + 1) * P],
                                rhs=w_tiles[2][:, kk, :],
                                start=(kk == 0),
                                stop=(kk == NKC - 1),
                            )
                        nc.vector.tensor_copy(
                            v_tile[:, tt, :, 0:HWID],
                            ps[:].rearrange("p (h w) -> p h w", h=HPC),
                        )

            # ================= attention phase =================
            with (
                tc.tile_pool(name="sc", bufs=2, space="PSUM") as scp,
                tc.tile_pool(name="ov", bufs=3, space="PSUM") as ovp,
                tc.tile_pool(name="tr", bufs=1, space="PSUM") as trp,
            ):
                for h in range(HPC):
                    sl = mm_slice(h)
                    for ip in range(2):  # i-window of 1024 = i-chunks (2ip, 2ip+1)
                        i_base = 1024 * ip
                        jmax = min(8 * ip + 7, NJ - 1)
                        otps = [
                            ovp.tile([HWID + 1, 512], F32, tag="ov", name=f"ov{h}_{ip}_{k2}")
                            for k2 in range(2)
                        ]
                        for J in range(jmax + 1):
                            dp = J - 8 * ip
                            c0 = max(0, 128 * dp)
                            ps = scp.tile([P, 1024], F32, tag="sc", name=f"sc{h}_{ip}_{J}")
                            # scores S^T[j, i] in psum-bank-sized segments
                            seg = c0
                            while seg < 1024:
                                send = min(1024, (seg // 512 + 1) * 512)
                                nc.tensor.matmul(
                                    ps[:, seg:send],
                                    lhsT=k_tiles[h][sl, J * P:(J + 1) * P],
                                    rhs=q_tiles[h][sl, i_base + seg:i_base + send],
                                    start=True,
                                    stop=True,
                                )
                                seg = send
                            pt = ptp.tile([P, 1024], F32R, tag="pt", name=f"pt{h}_{ip}_{J}")
                            nc.scalar.activation(
                                pt[:, c0:1024], ps[:, c0:1024],
                                mybir.ActivationFunctionType.Exp,
                                bias=ebias[:], scale=1.0,
                            )
                            if dp >= 0:
                                # mask the diagonal 128-strip
                                nc.vector.tensor_mul(
                                    pt[:, c0:c0 + P],
                                    pt[:, c0:c0 + P],
                                    tri_t[:],
                                )
                            for ii in range(2):
                                I = 2 * ip + ii
                                i0 = max(512 * I, 128 * J)
                                iend = 512 * I + 512
                                if i0 >= iend:
                                    continue
                                nc.tensor.matmul(
                                    otps[ii][:, i0 - 512 * I:512],
                                    lhsT=v_tile[:, J, h, :],
                                    rhs=pt[:, i0 - i_base:iend - i_base],
                                    start=(J == 0),
                                    stop=(J == min(4 * I + 3, jmax)),
                                )
                        # epilogue: transpose O^T, divide by denominator, store
                        for ii in range(2):
                            I = 2 * ip + ii
                            osb = otp_sb.tile([HWID + 1, 512], F32, tag="ot", name=f"ot{h}_{ip}_{ii}")
                            nc.vector.tensor_copy(osb[:], otps[ii][:])
                            for u in range(4):
                                otr = trp.tile([P, HWID + 1], F32, tag="tr", name=f"tr{h}_{ip}_{ii}_{u}")
                                nc.tensor.transpose(
                                    otr[:], osb[:, u * P:(u + 1) * P], ident[:]
                                )
                                rec = rcp.tile([P, 1], F32, tag="rc", name=f"rc{h}_{ip}_{ii}_{u}")
                                nc.vector.reciprocal(rec[:], otr[:, HWID:HWID + 1])
                                ob = obp.tile([P, HWID], F32, tag="ob", name=f"ob{h}_{ip}_{ii}_{u}")
                                nc.scalar.mul(ob[:], otr[:, 0:HWID], rec[:])
                                nc.sync.dma_start(
                                    out[512 * I + u * P:512 * I + (u + 1) * P,
                                        h * HWID:(h + 1) * HWID],
                                    ob[:],
                                )
    nc.compile()
    return nc


_NC = None


def _get_nc():
    global _NC
    if _NC is None:
        _NC = build_kernel()
    return _NC


def kernel(queries, keys, values, mask, Wq, Wk, Wv):
    B = queries.shape[0]
    asc = np.ascontiguousarray
    scale = 1.0 / np.sqrt(HWID)

    WqTs = asc((Wq.T * scale).astype(np.float32))
    WkT = asc(Wk.T.astype(np.float32))
    WvT = asc(Wv.T.astype(np.float32))
    qTs = [asc(queries[b].T.astype(np.float32)) for b in range(B)]
    kTs = [asc(keys[b].T.astype(np.float32)) for b in range(B)]
    vTs = [asc(values[b].T.astype(np.float32)) for b in range(B)]

    slopes = (2.0 ** (-np.arange(1, H + 1) * (8.0 / H))).astype(np.float32)
    iv = np.arange(S, dtype=np.float32)
    tri_np = np.asarray(
        np.arange(P)[:, None] <= np.arange(P)[None, :], dtype=np.float32
    )  # keep j<=i: rows p (j), cols u (i)

    nc = _get_nc()
    in_maps = []
    for c in range(8):
        b, g = divmod(c, 4)
        a = np.zeros((4 * HPC, S), np.float32)
        for hl in range(HPC):
            h = 4 * g + hl
            a[4 * hl + 0] = slopes[h]
            a[4 * hl + 1] = -slopes[h] * iv
            a[4 * hl + 2] = iv
            a[4 * hl + 3] = 1.0
        in_maps.append({
            "qT": qTs[b], "kT": kTs[b], "vT": vTs[b],
            "wqT": asc(WqTs[:, g * CW:(g + 1) * CW]),
            "wkT": asc(WkT[:, g * CW:(g + 1) * CW]),
            "wvT": asc(WvT[:, g * CW:(g + 1) * CW]),
            "aug": a, "tri": tri_np, "zs": np.zeros((62, S), np.float32), "on1": np.ones((P, 64), np.float32),
        })

    res = run_bass_kernel_spmd(nc, in_maps, core_ids=list(range(8)))
    outp = np.empty((B, S, D), np.float32)
    for c in range(8):
        b, g = divmod(c, 4)
        outp[b, :, g * CW:(g + 1) * CW] = res.results[c]["out"]
    return outp



# revision 21
# speedup vs baseline: 1.6528x; 1.6528x over previous
"""MHSA (dense transformer, ALiBi + causal) TRN2 Bass kernel, 8-core SPMD.

Sharding: batch (2) x head-group (4 groups of 4 heads) -> 8 cores.
Head groups are chosen so every core gets one head from each ALiBi band
class; per-head causal attention is BANDED: with slope s_h, weights decay
like e^{s_h (j-i)}, so blocks further than nb_h*128 below the diagonal
contribute < e^-15 relative mass and are skipped. Band slots (in local
head order) are [16, 9, 3, 2] diagonal 128-blocks, identical on every
core (uniform SPMD instruction stream).

Per core, for its batch b and 4 heads:
  Q^T = (Wq_g/8) @ X_q^T, K^T = Wk_g @ X_k^T   (bf16 matmuls, f32 psum)
    psum [128 = 2 heads, t] split to per-head tiles (f32r) with 2 exact
    aug rows folding ALiBi into the S contraction:
       q~ = [q; slope_h; -slope_h*i],  k~ = [k; j; 1]
    (contraction rows are free on the PE; cost is output columns only)
  S^T[j, i] banded, streamed in >=256-col chunks into [128,1024] psum
    windows; exp(S - 12) on ScalarE (wide calls), output bf16 pt tiles.
    The softmax shift is row-invariant so a constant shift suffices.
  V = X_v @ Wv_g^T  -> v~[j, d] bf16 with a ones column (row-sum ->
    softmax denominator).
  O[i, d] = sum_J pt_J^T @ v~_J : psum [128 i, 65]; col 64 is the
    denominator. DVE reciprocal + per-partition scale -> bf16 out,
    DMA per 128-row block; host casts to f32 and gathers head columns.
"""

import numpy as np
import ml_dtypes

import concourse.bass as bass
import concourse.mybir as mybir
import concourse.tile as tile
from concourse import bacc
from concourse.bass_utils import run_bass_kernel_spmd

P = 128
S = 2048
D = 1024
H = 16
HWID = 64
HPC = 4            # heads per core
CW = HPC * HWID    # 256 weight/output cols per core
NKC = D // P       # 8 contraction chunks
NJ = S // P        # 16 j/i blocks of 128

F32 = mybir.dt.float32
F32R = mybir.dt.float32r
BF16 = mybir.dt.bfloat16

EXP_BIAS = -12.0

# ALiBi band widths (in 128-blocks, incl. diagonal) per local head slot.
NB_SLOT = [16, 9, 3, 2]
# global head ids per (group, slot): slot0 gets the widest-band heads.
GROUPS = [[13, 11, 7, 3], [14, 10, 6, 2], [15, 9, 5, 1], [12, 8, 4, 0]]


def _seg_plan():
    """Pack each head's banded S^T segments into [128,1024] psum windows.

    Chunks split at 512 (psum bank) boundaries; a small alignment gap is
    inserted before some segments so nearly all chunks are >=256 wide
    (f32r matmuls run 4x slower below 256 output cols). Gap columns hold
    garbage, get exp'd, and are never read by the O matmuls.

    Returns per local head slot: ([(J, ev, flush_before, gap, pt_off)],
    pt_width).
    """
    plans = []
    for nb in NB_SLOT:
        segs = []
        ptw = 0   # pt col offset == cumulative window content (incl gaps)
        cur = 0   # fill position within current window
        for J in range(NJ):
            ev = 128 * min(nb, NJ - J)
            m = ev % 512
            targets = {0: [0], 128: [256], 256: [0, 256], 384: [0, 128]}[m]
            a0 = cur % 512
            gap = min(((t - a0) % 512) for t in targets)
            flush = False
            if cur + gap >= 1024:   # start a fresh window instead
                flush = True
                cur = 0
                gap = min(targets)
            segs.append((J, ev, flush, gap, ptw + gap))
            cur += gap
            ptw += gap
            rem = ev
            while rem:
                if cur == 1024:
                    cur = 0
                w = min(rem, 512 - (cur % 512))
                rem -= w
                cur += w
                ptw += w
        plans.append((segs, ptw))
    return plans


SEG_PLANS = _seg_plan()


def build_kernel():
    nc = bacc.Bacc("TRN2")

    xq = nc.dram_tensor("xq", [D, S], BF16, kind="ExternalInput")
    xk = nc.dram_tensor("xk", [D, S], BF16, kind="ExternalInput")
    xv = nc.dram_tensor("xv", [D, S], BF16, kind="ExternalInput")
    wq = nc.dram_tensor("wq", [D, CW], BF16, kind="ExternalInput")
    wk = nc.dram_tensor("wk", [D, CW], BF16, kind="ExternalInput")
    wv = nc.dram_tensor("wv", [D, CW], BF16, kind="ExternalInput")
    aug = nc.dram_tensor("aug", [4 * HPC, S], F32R, kind="ExternalInput")
    zs = nc.dram_tensor("zs", [HWID - 2, S], F32R, kind="ExternalInput")
    on1 = nc.dram_tensor("on1", [P, NJ * HPC], BF16, kind="ExternalInput")
    tri = nc.dram_tensor("tri", [P, P], BF16, kind="ExternalInput")
    out = nc.dram_tensor("out", [S, CW], BF16, kind="ExternalOutput")

    with tile.TileContext(nc) as tc:
        with (
            tc.tile_pool(name="qk", bufs=1) as qkp,
            tc.tile_pool(name="vv", bufs=1) as vvp,
            tc.tile_pool(name="ob", bufs=1) as obp,
            tc.tile_pool(name="rc", bufs=4) as rcp,
        ):
            # ---- long-lived SBUF ----
            q_t = [qkp.tile([P, S], F32R, tag=f"qh{h}", name=f"qh{h}") for h in range(HPC)]
            k_t = [qkp.tile([P, S], F32R, tag=f"kh{h}", name=f"kh{h}") for h in range(HPC)]
            v_sb = vvp.tile([P, NJ, HPC, HWID + 1], BF16, tag="v", name="v_sb")
            out_sb = obp.tile([P, NJ, CW], BF16, tag="ob", name="out_sb")
            ebias = vvp.tile([P, 1], F32, tag="ebias", name="ebias")

            # ---- scoped x/w pools (manual lifetimes) ----
            x2p = tc.alloc_tile_pool(name="x2", bufs=1, space="SBUF")   # wv, xv
            x1p = tc.alloc_tile_pool(name="x1", bufs=1, space="SBUF")   # wq, wk, xq, xk

            wvt = x2p.tile([P, NKC, CW], BF16, tag="wv", name="wv_t")
            xv_k = [x2p.tile([P, S], BF16, tag=f"xv{k}", name=f"xv{k}") for k in range(NKC)]
            wqt = x1p.tile([P, NKC, CW], BF16, tag="wq", name="wq_t")
            wkt = x1p.tile([P, NKC, CW], BF16, tag="wk", name="wk_t")
            xq_k = [x1p.tile([P, S], BF16, tag=f"xq{k}", name=f"xq{k}") for k in range(NKC)]
            xk_k = [x1p.tile([P, S], BF16, tag=f"xk{k}", name=f"xk{k}") for k in range(NKC)]

            # DMA order: QK weights+inputs first (ko-interleaved), then aug,
            # then V weights+inputs.
            for k in range(NKC):
                nc.sync.dma_start(
                    wqt[:, k, :], wq.rearrange("(ko p) c -> p ko c", p=P)[:, k, :])
                nc.sync.dma_start(xq_k[k][:], xq[k * P:(k + 1) * P, :])
                nc.sync.dma_start(
                    wkt[:, k, :], wk.rearrange("(ko p) c -> p ko c", p=P)[:, k, :])
                nc.sync.dma_start(xk_k[k][:], xk[k * P:(k + 1) * P, :])

            # per-head q~/k~ aug rows:
            # even local head: data rows 0:64, aug rows 64:66, mm slice 0:66
            # odd  local head: zeros 0:62, aug 62:64, data 64:128, slice 0:128
            def aug_row(h):
                return HWID if h % 2 == 0 else HWID - 2

            def mm_slice(h):
                return slice(0, HWID + 2) if h % 2 == 0 else slice(0, P)

            for h in range(HPC):
                ra = aug_row(h)
                nc.sync.dma_start(q_t[h][ra:ra + 2, :], aug[4 * h:4 * h + 2, :])
                nc.sync.dma_start(k_t[h][ra:ra + 2, :], aug[4 * h + 2:4 * h + 4, :])

            for k in range(NKC):
                nc.sync.dma_start(
                    wvt[:, k, :], wv.rearrange("(ko p) c -> p ko c", p=P)[:, k, :])
                nc.sync.dma_start(xv_k[k][:], xv[k * P:(k + 1) * P, :])

            # zero unused rows of odd-head tiles; ones col; exp bias const
            for h in range(1, HPC, 2):
                nc.sync.dma_start(q_t[h][0:HWID - 2, :], zs[:])
                nc.sync.dma_start(k_t[h][0:HWID - 2, :], zs[:])
            nc.sync.dma_start(
                v_sb[:, :, :, HWID], on1.rearrange("p (a b) -> p a b", a=NJ))
            tri_t = vvp.tile([P, P], BF16, tag="tri", name="tri_t")
            nc.sync.dma_start(tri_t[:], tri[:])
            nc.gpsimd.memset(ebias[:], EXP_BIAS)

            # ---- pt (exp'd scores) per head, bf16 ----
            pt_t = [None] * HPC

            def qk_proj(pp, cc):
                """Project Q^T,K^T for head pair (2cc, 2cc+1): 4 t-blocks."""
                for tb in range(4):
                    tsl = slice(tb * 512, (tb + 1) * 512)
                    for wt, xs, dsts in (
                        (wqt, xq_k, q_t),
                        (wkt, xk_k, k_t),
                    ):
                        ps = pp.tile([P, 512], F32, tag="pp", name=f"pp{cc}_{tb}")
                        for kk in range(NKC):
                            nc.tensor.matmul(
                                ps[:],
                                lhsT=wt[:, kk, cc * P:(cc + 1) * P],
                                rhs=xs[kk][:, tsl],
                                start=(kk == 0),
                                stop=(kk == NKC - 1),
                            )
                        # rows 0:64 -> even head (ACT), 64:128 -> odd (DVE)
                        nc.scalar.copy(dsts[2 * cc][0:HWID, tsl], ps[0:HWID, :])
                        nc.vector.tensor_copy(dsts[2 * cc + 1][HWID:P, tsl], ps[HWID:P, :])

            def s_phase(scp, h):
                """Banded S^T + exp for head h via [128,1024] psum windows."""
                segs, _ = SEG_PLANS[h]
                sl = mm_slice(h)
                state = {"win": None, "cur": 0, "base": 0, "widx": 0, "diag": []}

                def flush():
                    if state["win"] is not None and state["cur"] > 0:
                        b0 = state["base"]
                        nc.scalar.activation(
                            pt_t[h][:, b0:b0 + state["cur"]],
                            state["win"][:, 0:state["cur"]],
                            mybir.ActivationFunctionType.Exp,
                            bias=ebias[:], scale=1.0,
                        )
                        state["base"] = b0 + state["cur"]
                        # causal mask for the diagonal 128-block of each
                        # segment in this window (zero j > i)
                        for o in state["diag"]:
                            nc.vector.tensor_mul(
                                pt_t[h][:, o:o + P], pt_t[h][:, o:o + P], tri_t[:])
                        state["diag"] = []
                    state["win"] = None
                    state["cur"] = 0

                def new_win():
                    flush()
                    state["win"] = scp.tile(
                        [P, 1024], F32, tag="sc", name=f"sc{h}_{state['widx']}")
                    state["widx"] += 1

                for J, ev, fl, gap, off in segs:
                    if fl:
                        flush()
                    if state["win"] is None:
                        new_win()
                    state["cur"] += gap   # alignment gap (garbage cols)
                    state["diag"].append(off)
                    # stream i in [128J, 128J+ev) in chunks split at 512
                    # boundaries of the psum window; a segment may span
                    # multiple windows.
                    done = 0
                    while done < ev:
                        if state["cur"] == 1024:
                            new_win()
                        cur = state["cur"]
                        w = min(ev - done, 512 - (cur % 512))
                        nc.tensor.matmul(
                            state["win"][:, cur:cur + w],
                            lhsT=k_t[h][sl, J * P:(J + 1) * P],
                            rhs=q_t[h][sl, J * P + done:J * P + done + w],
                            start=True,
                            stop=True,
                        )
                        done += w
                        state["cur"] = cur + w
                flush()

            def v_proj(pp):
                for tt in range(NJ):
                    ps = pp.tile([P, 512], F32, tag="pp", name=f"ppv{tt}")
                    for kk in range(NKC):
                        nc.tensor.matmul(
                            ps[:, 0:CW],
                            lhsT=xv_k[kk][:, tt * P:(tt + 1) * P],
                            rhs=wvt[:, kk, :],
                            start=(kk == 0),
                            stop=(kk == NKC - 1),
                        )
                    nc.vector.tensor_copy(
                        v_sb[:, tt, :, 0:HWID],
                        ps[:, 0:CW].rearrange("p (h w) -> p h w", h=HPC),
                    )

            def o_phase(ovp, h, GSZ=6):
                """O[i,d] += pt_J^T @ v~_J, then divide+stage output."""
                segs, _ = SEG_PLANS[h]
                nb = NB_SLOT[h]
                ngrp = (NJ + GSZ - 1) // GSZ
                ogs = [
                    ovp.tile([P, GSZ, HWID + 1], F32, tag="ov", name=f"ov{h}_{g}")
                    for g in range(ngrp)
                ]
                # I-major: a matmul's start=True clears has_written for the
                # whole psum BANK, so each I group must fully finish before
                # the next group in the same bank starts.
                offs = {J: off for J, ev, fl, gap, off in segs}
                for I in range(NJ):
                    Jlo = max(0, I - nb + 1)
                    og = ogs[I // GSZ]
                    for J in range(Jlo, I + 1):
                        nc.tensor.matmul(
                            og[:, I % GSZ, :],
                            lhsT=pt_t[h][:, offs[J] + (I - J) * P: offs[J] + (I - J + 1) * P],
                            rhs=v_sb[:, J, h, :],
                            start=(J == Jlo),
                            stop=(J == I),
                        )
                # epilogue: divide by denominator (col 64), stage bf16
                for I in range(NJ):
                    og = ogs[I // GSZ]
                    rec = rcp.tile([P, 1], F32, tag="rc", name=f"rc{h}_{I}")
                    nc.vector.reciprocal(rec[:], og[:, I % GSZ, HWID:HWID + 1])
                    nc.vector.tensor_scalar_mul(
                        out_sb[:, I, h * HWID:(h + 1) * HWID],
                        og[:, I % GSZ, 0:HWID],
                        rec[:],
                    )

            scp = tc.alloc_tile_pool(name="sc", bufs=2, space="PSUM")
            pp = tc.alloc_tile_pool(name="pp", bufs=3, space="PSUM")

            qk_proj(pp, 0)      # heads 0,1
            qk_proj(pp, 1)      # heads 2,3
            x1p.release()

            ptp = tc.alloc_tile_pool(name="pt", bufs=1, space="SBUF")
            for h in range(HPC):
                pt_t[h] = ptp.tile(
                    [P, SEG_PLANS[h][1]], BF16, tag=f"pt{h}", name=f"pt{h}")

            s_phase(scp, 0)     # widest-band head first: exp overlaps V/S/O
            v_proj(pp)
            s_phase(scp, 1)
            s_phase(scp, 2)
            s_phase(scp, 3)
            pp.release()

            ovp = tc.alloc_tile_pool(name="ov", bufs=3, space="PSUM")
            for h in range(HPC):
                o_phase(ovp, h)

            for I in range(NJ):
                nc.sync.dma_start(out[I * P:(I + 1) * P, :], out_sb[:, I, :])

            ovp.release()
            scp.release()
            ptp.release()
            x2p.release()

    nc.compile()
    return nc


_NC = None


def _get_nc():
    global _NC
    if _NC is None:
        _NC = build_kernel()
    return _NC


def kernel(queries, keys, values, mask, Wq, Wk, Wv):
    B = queries.shape[0]
    bf16 = ml_dtypes.bfloat16
    asc = np.ascontiguousarray
    scale = 1.0 / np.sqrt(HWID)

    WqTs = (Wq.T * scale).astype(np.float32)
    WkT = Wk.T.astype(np.float32)
    WvT = Wv.T.astype(np.float32)
    xqs = [asc(queries[b].T).astype(bf16) for b in range(B)]
    xks = [asc(keys[b].T).astype(bf16) for b in range(B)]
    xvs = [asc(values[b].T).astype(bf16) for b in range(B)]

    slopes = (2.0 ** (-np.arange(1, H + 1) * (8.0 / H))).astype(np.float32)
    iv = np.arange(S, dtype=np.float32)
    # keep j <= i: rows p (j within block), cols u (i within block)
    tri_np = np.asarray(
        np.arange(P)[:, None] <= np.arange(P)[None, :], dtype=np.float32
    ).astype(bf16)

    nc = _get_nc()
    in_maps = []
    for c in range(8):
        b, g = divmod(c, 4)
        heads = GROUPS[g]
        cols = np.concatenate([np.arange(h * HWID, (h + 1) * HWID) for h in heads])
        a = np.zeros((4 * HPC, S), np.float32)
        for hl, h in enumerate(heads):
            a[4 * hl + 0] = slopes[h]
            a[4 * hl + 1] = -slopes[h] * iv
            a[4 * hl + 2] = iv
            a[4 * hl + 3] = 1.0
        in_maps.append({
            "xq": xqs[b], "xk": xks[b], "xv": xvs[b],
            "wq": asc(WqTs[:, cols]).astype(bf16),
            "wk": asc(WkT[:, cols]).astype(bf16),
            "wv": asc(WvT[:, cols]).astype(bf16),
            "aug": a,
            "zs": np.zeros((HWID - 2, S), np.float32),
            "on1": np.ones((P, NJ * HPC), bf16),
            "tri": tri_np,
        })

    res = run_bass_kernel_spmd(nc, in_maps, core_ids=list(range(8)))
    outp = np.empty((B, S, D), np.float32)
    for c in range(8):
        b, g = divmod(c, 4)
        heads = GROUPS[g]
        o = np.asarray(res.results[c]["out"]).astype(np.float32)
        for hl, h in enumerate(heads):
            outp[b, :, h * HWID:(h + 1) * HWID] = o[:, hl * HWID:(hl + 1) * HWID]
    return outp


# revision 25
# speedup vs baseline: 1.9293x; 1.1673x over previous
"""MHSA (dense transformer, ALiBi + causal) TRN2 Bass kernel, 8-core SPMD.

Sharding: batch (2) x head-group (4 groups of 4 heads) -> 8 cores.
Head groups are chosen so every core gets one head from each ALiBi band
class; per-head causal attention is BANDED: with slope s_h, weights decay
like e^{s_h (j-i)}, so blocks further than nb_h*128 below the diagonal
contribute < e^-15 relative mass and are skipped. Band slots (in local
head order) are [16, 9, 3, 2] diagonal 128-blocks, identical on every
core (uniform SPMD instruction stream).

Per core, for its batch b and 4 heads:
  Q^T = (Wq_g/8) @ X_q^T, K^T = Wk_g @ X_k^T   (bf16 matmuls, f32 psum)
    kk-outer accumulation, two t-half passes over 4 [128,1024] psum
    tiles (8 banks) so the PE trails the half-split x DMA stream; psum
    [128 = 2 heads, t] split to per-head tiles (f32r) with 2 exact aug
    rows folding ALiBi into the S contraction:
       q~ = [q; slope_h; -slope_h*i],  k~ = [k; j; 1]
    (contraction rows are free on the PE; cost is output columns only)
  S^T[j, i] banded, streamed into [128,1024] psum windows; exp(S - 12)
    on ScalarE (wide calls), bf16 pt tiles; diagonal 128-blocks masked
    with a lower-triangular multiply on DVE. The softmax shift is
    row-invariant so a constant shift suffices.
  V = X_v @ Wv_g^T  -> v~[j, d] bf16 with a ones column (row-sum ->
    softmax denominator).
  O[i, d] = sum_J pt_J^T @ v~_J : psum [128 i, 65]; col 64 is the
    denominator. I-major group order (a matmul's start clears the whole
    psum bank's has_written bits). DVE reciprocal + per-partition scale
    -> bf16, one DMA per head; host casts to f32 and gathers.

Emission interleaves S windows with V-projection and O "filler" work so
the in-order PE stream never head-of-line blocks on the exp (ACT) pipe.
"""

import numpy as np
import ml_dtypes

import concourse.bass as bass
import concourse.mybir as mybir
import concourse.tile as tile
from concourse import bacc
from concourse.bass_utils import run_bass_kernel_spmd

P = 128
S = 2048
D = 1024
H = 16
HWID = 64
HPC = 4            # heads per core
CW = HPC * HWID    # 256 weight/output cols per core
NKC = D // P       # 8 contraction chunks
NJ = S // P        # 16 j/i blocks of 128
HS = S // 2        # t-half

F32 = mybir.dt.float32
F32R = mybir.dt.float32r
BF16 = mybir.dt.bfloat16

EXP_BIAS = -12.0

# ALiBi band widths (in 128-blocks, incl. diagonal) per local head slot.
NB_SLOT = [16, 9, 3, 2]
# global head ids per (group, slot): slot0 gets the widest-band heads.
GROUPS = [[13, 11, 7, 3], [14, 10, 6, 2], [15, 9, 5, 1], [12, 8, 4, 0]]


def _seg_plan():
    """Pack each head's banded S^T segments into [128,1024] psum windows.

    Returns per local head slot (wins, offs, ptw):
      wins: list of (width, pt_base, chunks, diags); chunks are
            (J, j_done, win_off, w) matmuls split at 512 (psum bank)
            boundaries; diags are pt offsets of diagonal 128-blocks to
            tri-mask after the window's exp.
      offs: {J: pt col offset of segment J};  ptw: total pt width.
    """
    plans = []
    for nb in NB_SLOT:
        wins = []
        offs = {}
        state = {"chunks": [], "diags": [], "cur": 0, "ptbase": 0}
        ptw = 0

        def close(state=state):
            if state["cur"] > 0:
                wins.append(
                    (state["cur"], state["ptbase"], state["chunks"], state["diags"]))
                state["ptbase"] += state["cur"]
            state["chunks"] = []
            state["diags"] = []
            state["cur"] = 0

        for J in range(NJ):
            ev = 128 * min(nb, NJ - J)
            offs[J] = ptw
            if state["cur"] == 1024:
                close()
            state["diags"].append(ptw)
            done = 0
            while done < ev:
                if state["cur"] == 1024:
                    close()
                w = min(ev - done, 512 - (state["cur"] % 512))
                state["chunks"].append((J, done, state["cur"], w))
                done += w
                state["cur"] += w
                ptw += w
        close()
        plans.append((wins, offs, ptw))
    return plans


SEG_PLANS = _seg_plan()


def build_kernel():
    nc = bacc.Bacc("TRN2")

    xq = nc.dram_tensor("xq", [D, S], BF16, kind="ExternalInput")
    xk = nc.dram_tensor("xk", [D, S], BF16, kind="ExternalInput")
    xv = nc.dram_tensor("xv", [D, S], BF16, kind="ExternalInput")
    wq = nc.dram_tensor("wq", [D, CW], BF16, kind="ExternalInput")
    wk = nc.dram_tensor("wk", [D, CW], BF16, kind="ExternalInput")
    wv = nc.dram_tensor("wv", [D, CW], BF16, kind="ExternalInput")
    aug = nc.dram_tensor("aug", [4 * HPC, S], F32R, kind="ExternalInput")
    zs = nc.dram_tensor("zs", [HWID - 2, S], F32R, kind="ExternalInput")
    on1 = nc.dram_tensor("on1", [P, NJ * HPC], BF16, kind="ExternalInput")
    tri = nc.dram_tensor("tri", [P, P], BF16, kind="ExternalInput")
    outs_d = [
        nc.dram_tensor(f"out{h}", [S, HWID], BF16, kind="ExternalOutput")
        for h in range(HPC)
    ]

    with tile.TileContext(nc) as tc:
        with (
            tc.tile_pool(name="qk", bufs=1) as qkp,
            tc.tile_pool(name="vv", bufs=1) as vvp,
            tc.tile_pool(name="ob", bufs=1) as obp,
            tc.tile_pool(name="rc", bufs=4) as rcp,
        ):
            # ---- long-lived SBUF ----
            q_t = [qkp.tile([P, S], F32R, tag=f"qh{h}", name=f"qh{h}") for h in range(HPC)]
            k_t = [qkp.tile([P, S], F32R, tag=f"kh{h}", name=f"kh{h}") for h in range(HPC)]
            v_sb = vvp.tile([P, NJ, HPC, HWID + 1], BF16, tag="v", name="v_sb")
            out_sb = [
                obp.tile([P, NJ, HWID], BF16, tag=f"ob{h}", name=f"ob{h}")
                for h in range(HPC)
            ]
            ebias = vvp.tile([P, 1], F32, tag="ebias", name="ebias")
            tri_t = vvp.tile([P, P], BF16, tag="tri", name="tri_t")

            # ---- scoped x/w pools ----
            x2p = tc.alloc_tile_pool(name="x2", bufs=1, space="SBUF")   # wv, xv
            x1p = tc.alloc_tile_pool(name="x1", bufs=1, space="SBUF")   # wq, wk, xq, xk

            wvt = x2p.tile([P, NKC, CW], BF16, tag="wv", name="wv_t")
            xv_k = [[x2p.tile([P, HS], BF16, tag=f"xv{k}_{hf}", name=f"xv{k}_{hf}")
                     for hf in range(2)] for k in range(NKC)]
            wqt = x1p.tile([P, NKC, CW], BF16, tag="wq", name="wq_t")
            wkt = x1p.tile([P, NKC, CW], BF16, tag="wk", name="wk_t")
            xq_k = [[x1p.tile([P, HS], BF16, tag=f"xq{k}_{hf}", name=f"xq{k}_{hf}")
                     for hf in range(2)] for k in range(NKC)]
            xk_k = [[x1p.tile([P, HS], BF16, tag=f"xk{k}_{hf}", name=f"xk{k}_{hf}")
                     for hf in range(2)] for k in range(NKC)]

            # DMA order: QK weights, (xq,xk) t-half 0, consts, t-half 1,
            # then wv + xv half 0, zeros, xv half 1.
            nc.sync.dma_start(wqt[:], wq.rearrange("(ko p) c -> p ko c", p=P))
            nc.sync.dma_start(wkt[:], wk.rearrange("(ko p) c -> p ko c", p=P))

            def aug_row(h):
                return HWID if h % 2 == 0 else HWID - 2

            def mm_slice(h):
                return slice(0, HWID + 2) if h % 2 == 0 else slice(0, P)

            def emit_half_dmas(hf):
                for k in range(NKC):
                    rsl = slice(k * P, (k + 1) * P)
                    csl = slice(hf * HS, (hf + 1) * HS)
                    nc.sync.dma_start(xq_k[k][hf][:], xq[rsl, csl])
                    nc.sync.dma_start(xk_k[k][hf][:], xk[rsl, csl])

            emit_half_dmas(0)
            # small constants between the big x streams
            # even local head: data rows 0:64, aug rows 64:66, mm slice 0:66
            # odd  local head: zeros 0:62, aug 62:64, data 64:128, slice 0:128
            for h in range(HPC):
                ra = aug_row(h)
                nc.sync.dma_start(q_t[h][ra:ra + 2, :], aug[4 * h:4 * h + 2, :])
                nc.sync.dma_start(k_t[h][ra:ra + 2, :], aug[4 * h + 2:4 * h + 4, :])
            nc.sync.dma_start(
                v_sb[:, :, :, HWID], on1.rearrange("p (a b) -> p a b", a=NJ))
            nc.sync.dma_start(tri_t[:], tri[:])
            nc.gpsimd.memset(ebias[:], EXP_BIAS)
            emit_half_dmas(1)

            nc.sync.dma_start(wvt[:], wv.rearrange("(ko p) c -> p ko c", p=P))
            for k in range(NKC):
                nc.sync.dma_start(xv_k[k][0][:], xv[k * P:(k + 1) * P, 0:HS])
            # zero rows 0:62 of odd-head q~/k~ tiles: one DMA + gpsimd copies
            nc.sync.dma_start(k_t[1][0:HWID - 2, :], zs[:])
            for k in range(NKC):
                nc.sync.dma_start(xv_k[k][1][:], xv[k * P:(k + 1) * P, HS:S])
            for dst in (k_t[3], q_t[1], q_t[3]):
                nc.gpsimd.tensor_copy(dst[0:HWID - 2, :], k_t[1][0:HWID - 2, :])

            # ---------- phase 1: QK projection (both cc), t-half passes ----
            # 4 psum tiles [128, 1024] = 8 banks; kk-outer; per kk-round 8
            # matmuls (4 tiles x 2 column halves of 512).
            ppQK = tc.alloc_tile_pool(name="pq", bufs=4, space="PSUM")
            for hf in range(2):
                pst = {}
                for cc in range(2):
                    for pj, (wt, xs) in enumerate(((wqt, xq_k), (wkt, xk_k))):
                        pst[(pj, cc)] = ppQK.tile(
                            [P, 1024], F32, tag="pp", name=f"p{hf}_{pj}{cc}")
                for kk in range(NKC):
                    for cc in range(2):
                        for pj, (wt, xs) in enumerate(((wqt, xq_k), (wkt, xk_k))):
                            ps = pst[(pj, cc)]
                            for ch in range(2):
                                nc.tensor.matmul(
                                    ps[:, ch * 512:(ch + 1) * 512],
                                    lhsT=wt[:, kk, cc * P:(cc + 1) * P],
                                    rhs=xs[kk][hf][:, ch * 512:(ch + 1) * 512],
                                    start=(kk == 0), stop=(kk == NKC - 1))
                for cc in range(2):
                    for pj, dsts in enumerate((q_t, k_t)):
                        ps = pst[(pj, cc)]
                        tsl = slice(hf * HS, (hf + 1) * HS)
                        # rows 0:64 -> even head (ACT), 64:128 -> odd (DVE)
                        nc.scalar.copy(dsts[2 * cc][0:HWID, tsl], ps[0:HWID, :])
                        nc.vector.tensor_copy(
                            dsts[2 * cc + 1][HWID:P, tsl], ps[HWID:P, :])
            ppQK.release()
            x1p.release()

            # ---------- pools for the interleaved attention stream --------
            ptp = tc.alloc_tile_pool(name="pt", bufs=1, space="SBUF")
            pt_t = [
                ptp.tile([P, SEG_PLANS[h][2]], BF16, tag=f"pt{h}", name=f"pt{h}")
                for h in range(HPC)
            ]
            scp = tc.alloc_tile_pool(name="sc", bufs=2, space="PSUM")
            ppR = tc.alloc_tile_pool(name="pr", bufs=4, space="PSUM")

            # ---- filler emitters (PE work between S windows) ----
            def v_proj_fillers():
                for tt in range(NJ):
                    hf, to = divmod(tt, NJ // 2)
                    def one(tt=tt, hf=hf, to=to):
                        ps = ppR.tile([P, 512], F32, tag="pr", name=f"v{tt}")
                        for kk in range(NKC):
                            nc.tensor.matmul(
                                ps[:, 0:CW],
                                lhsT=xv_k[kk][hf][:, to * P:(to + 1) * P],
                                rhs=wvt[:, kk, :],
                                start=(kk == 0), stop=(kk == NKC - 1))
                        nc.vector.tensor_copy(
                            v_sb[:, tt, :, 0:HWID],
                            ps[:, 0:CW].rearrange("p (h w) -> p h w", h=HPC))
                    # xv half 0 lands ~34us, half 1 ~43us (est)
                    yield one, 860, 34500.0 if hf == 0 else 43500.0

            def o_fillers(h, ovp, GSZ=6):
                _, offs, _ = SEG_PLANS[h]
                nb = NB_SLOT[h]
                ngrp = (NJ + GSZ - 1) // GSZ
                ogs = [ovp.tile([P, GSZ, HWID + 1], F32, tag="ov", name=f"ov{h}_{g}")
                       for g in range(ngrp)]
                # I-major: a matmul's start=True clears has_written for the
                # whole psum BANK, so each I group must fully finish before
                # the next group in the same bank starts.
                for I in range(NJ):
                    def one(I=I):
                        Jlo = max(0, I - nb + 1)
                        og = ogs[I // GSZ]
                        for J in range(Jlo, I + 1):
                            o = offs[J]
                            nc.tensor.matmul(
                                og[:, I % GSZ, :],
                                lhsT=pt_t[h][:, o + (I - J) * P: o + (I - J + 1) * P],
                                rhs=v_sb[:, J, h, :],
                                start=(J == Jlo), stop=(J == I))
                    yield one, min(nb, 16) * 30 + 40, 0.0
                def epi():
                    for I in range(NJ):
                        og = ogs[I // GSZ]
                        rec = rcp.tile([P, 1], F32, tag="rc", name=f"rc{h}_{I}")
                        nc.vector.reciprocal(rec[:], og[:, I % GSZ, HWID:HWID + 1])
                        nc.vector.tensor_scalar_mul(
                            out_sb[h][:, I, :], og[:, I % GSZ, 0:HWID], rec[:])
                    nc.sync.dma_start(
                        outs_d[h].rearrange("(a p) b -> p a b", p=P), out_sb[h][:])
                yield epi, 10, 0.0

            # ---- S window emitter ----
            def emit_win(h, win, widx):
                width, ptbase, chunks, diags = win
                sl = mm_slice(h)
                w_t = scp.tile([P, 1024], F32, tag="sc", name=f"sc{h}_{widx}")
                for J, jd, wo, w in chunks:
                    nc.tensor.matmul(
                        w_t[:, wo:wo + w],
                        lhsT=k_t[h][sl, J * P:(J + 1) * P],
                        rhs=q_t[h][sl, J * P + jd:J * P + jd + w],
                        start=True, stop=True)
                nc.scalar.activation(
                    pt_t[h][:, ptbase:ptbase + width], w_t[:, 0:width],
                    mybir.ActivationFunctionType.Exp, bias=ebias[:], scale=1.0)
                for o in diags:
                    nc.vector.tensor_mul(
                        pt_t[h][:, o:o + P], pt_t[h][:, o:o + P], tri_t[:])

            # ---- interleaved emission: S windows drive; fillers keep PE
            # fed while ACT chews exp. Absolute-time estimates (ns).
            pe_abs = 29500.0   # ~end of phase 1 on PE
            act_abs = 29500.0
            filler_queues = [list(v_proj_fillers())]
            ovp = None

            def run_fillers():
                nonlocal pe_abs
                while filler_queues:
                    q = filler_queues[0]
                    if not q:
                        filler_queues.pop(0)
                        continue
                    fn, pe_ns, avail = q[0]
                    if pe_abs >= act_abs:
                        break
                    if avail > pe_abs:
                        # head-of-line not ready; don't risk a PE stall
                        break
                    q.pop(0)
                    fn()
                    pe_abs += pe_ns
                return

            def drain_fillers(n_queues=None):
                nonlocal pe_abs
                cnt = len(filler_queues) if n_queues is None else n_queues
                while cnt > 0 and filler_queues:
                    q = filler_queues[0]
                    if not q:
                        filler_queues.pop(0)
                        cnt -= 1
                        continue
                    fn, pe_ns, avail = q.pop(0)
                    fn()
                    pe_abs += max(pe_ns, avail - pe_abs if avail > pe_abs else pe_ns)

            for h in range(HPC):
                wins, offs, ptw = SEG_PLANS[h]
                for widx, win in enumerate(wins):
                    emit_win(h, win, widx)
                    pe_abs += win[0] * 0.4167
                    act_abs = max(act_abs, pe_abs) + win[0] * 0.833 + 190
                    run_fillers()
                if h == 1:
                    # V must drain before the psum pool swap for O
                    drain_fillers(1)
                    ppR.release()
                    ovp = tc.alloc_tile_pool(name="ov", bufs=3, space="PSUM")
                if h >= 1:
                    filler_queues.append(list(o_fillers(h - 1, ovp)))
            drain_fillers()
            for fn, _, _ in o_fillers(HPC - 1, ovp):
                fn()

            ovp.release()
            scp.release()
            ptp.release()
            x2p.release()

    nc.compile()
    return nc


_NC = None


def _get_nc():
    global _NC
    if _NC is None:
        _NC = build_kernel()
    return _NC


def kernel(queries, keys, values, mask, Wq, Wk, Wv):
    B = queries.shape[0]
    bf16 = ml_dtypes.bfloat16
    asc = np.ascontiguousarray
    scale = 1.0 / np.sqrt(HWID)

    WqTs = (Wq.T * scale).astype(np.float32)
    WkT = Wk.T.astype(np.float32)
    WvT = Wv.T.astype(np.float32)
    xqs = [asc(queries[b].T).astype(bf16) for b in range(B)]
    xks = [asc(keys[b].T).astype(bf16) for b in range(B)]
    xvs = [asc(values[b].T).astype(bf16) for b in range(B)]

    slopes = (2.0 ** (-np.arange(1, H + 1) * (8.0 / H))).astype(np.float32)
    iv = np.arange(S, dtype=np.float32)
    # keep j <= i: rows p (j within block), cols u (i within block)
    tri_np = np.asarray(
        np.arange(P)[:, None] <= np.arange(P)[None, :], dtype=np.float32
    ).astype(bf16)

    nc = _get_nc()
    in_maps = []
    for c in range(8):
        b, g = divmod(c, 4)
        heads = GROUPS[g]
        cols = np.concatenate([np.arange(h * HWID, (h + 1) * HWID) for h in heads])
        a = np.zeros((4 * HPC, S), np.float32)
        for hl, h in enumerate(heads):
            a[4 * hl + 0] = slopes[h]
            a[4 * hl + 1] = -slopes[h] * iv
            a[4 * hl + 2] = iv
            a[4 * hl + 3] = 1.0
        in_maps.append({
            "xq": xqs[b], "xk": xks[b], "xv": xvs[b],
            "wq": asc(WqTs[:, cols]).astype(bf16),
            "wk": asc(WkT[:, cols]).astype(bf16),
            "wv": asc(WvT[:, cols]).astype(bf16),
            "aug": a,
            "zs": np.zeros((HWID - 2, S), np.float32),
            "on1": np.ones((P, NJ * HPC), bf16),
            "tri": tri_np,
        })

    res = run_bass_kernel_spmd(nc, in_maps, core_ids=list(range(8)))
    outp = np.empty((B, S, D), np.float32)
    for c in range(8):
        b, g = divmod(c, 4)
        heads = GROUPS[g]
        for hl, h in enumerate(heads):
            o = np.asarray(res.results[c][f"out{hl}"]).astype(np.float32)
            outp[b, :, h * HWID:(h + 1) * HWID] = o
    return outp
